# revision 10
# baseline (speedup 1.0000x reference)
"""APPNP conv kernel for 8 TRN2 NeuronCores.

out = 0.8 * spmm(adj, h) + 0.2 * h0
  spmm: out[i] = sum_{e: row[e]==i} vals[e] * h[col[e]],  N=100000, E=1.6M, d=64

Per core (nodes row-partitioned 12500/core, no collectives):
  - host: permute nodes into 32-node windows with ~equal edge counts;
    4 window-groups per core; per group, distinct cols are remapped to
    compact ids and h rows (bf16) are packed into an SBUF-resident pair
    table (256B elements = 2 rows, tpr=64 two-stripe layout).
  - device: dma_gather (SBUF source, transposed) pulls each edge's pair
    element; PE transposes 128-slot chunks back to edge-major; PE reduces
    each 128-edge tile into its window's PSUM rows via two skinny
    val-carrying selection matmuls (even/odd pair half);
    eviction computes 0.8*psum + 0.2*h0.
"""
import sys
sys.path.insert(0, "/opt/trn_rl_repo")

import numpy as np
import ml_dtypes

import concourse.bacc as bacc
import concourse.bass as bass
import concourse.mybir as mybir
from concourse import bass_utils
from concourse.library_config import mlp
from concourse._compat import cdiv

N_NODES = 100000
ALPHA = 0.2
D = 64
CORES = 8
WIN = 32                 # nodes per window (matmul M)
BATCH_TILES = 32         # tiles per dma_gather instruction
NGRP = 4                 # window groups (tables) per core
TBL_PAIR_CAP = 20480     # pair rows per table (ids <= 40960)
TPR = 128


# ----------------------------------------------------------------- host prep
def _preprocess(edge_row, edge_col, edge_vals, h, h0):
    npc = N_NODES // CORES                      # 12500
    npc_pad = cdiv(npc, 128) * 128              # 12544
    nblocks = npc_pad // 128                    # 98
    nwin = npc_pad // WIN                       # 392
    wpg = cdiv(nwin, NGRP)                      # 98

    h_bf = np.asarray(h, dtype=np.float32).astype(ml_dtypes.bfloat16)
    edge_row = np.asarray(edge_row)
    edge_col = np.asarray(edge_col)
    edge_vals = np.asarray(edge_vals, dtype=np.float32)
    h0 = np.asarray(h0, dtype=np.float32)

    core_lo = np.searchsorted(edge_row, np.arange(CORES) * npc)
    core_hi = np.searchsorted(edge_row, (np.arange(CORES) + 1) * npc)

    cores = []
    for k in range(CORES):
        lo, hi = int(core_lo[k]), int(core_hi[k])
        rows = edge_row[lo:hi] - k * npc
        cols = edge_col[lo:hi]
        vals = edge_vals[lo:hi]

        deg = np.bincount(rows, minlength=npc_pad)
        order = np.argsort(-deg, kind="stable")
        half = npc_pad // 2
        perm = np.empty(npc_pad, dtype=np.int64)
        perm[0::2] = order[:half]
        perm[1::2] = order[half:][::-1]
        slot_of = np.empty(npc_pad, dtype=np.int64)
        slot_of[perm] = np.arange(npc_pad)

        eslot = slot_of[rows]
        eorder = np.argsort(eslot, kind="stable")
        e_slot = eslot[eorder]
        e_col = cols[eorder]
        e_val = vals[eorder]
        e_win = e_slot // WIN
        win_lo = np.searchsorted(e_win, np.arange(nwin))
        win_hi = np.searchsorted(e_win, np.arange(nwin) + 1)
        cores.append(dict(perm=perm, e_slot=e_slot, e_col=e_col, e_val=e_val,
                          win_lo=win_lo, win_hi=win_hi))

    s_w = np.ones(nwin, dtype=np.int64)
    for c in cores:
        s_w = np.maximum(s_w, cdiv(c["win_hi"] - c["win_lo"], 128))
    grp_wins = [np.arange(g * wpg, min((g + 1) * wpg, nwin)) for g in range(NGRP)]
    grp_tiles = [int(s_w[ws].sum()) for ws in grp_wins]
    batches = []
    for g in range(NGRP):
        left = grp_tiles[g]
        while left > 0:
            n = min(BATCH_TILES, left)
            batches.append((g, n))
            left -= n
    T = sum(grp_tiles)
    tile_win = np.concatenate([np.repeat(ws, s_w[ws]) for ws in grp_wins])
    assert len(tile_win) == T

    grp_of_win = np.zeros(nwin, dtype=np.int64)
    for g, ws in enumerate(grp_wins):
        grp_of_win[ws] = g

    ranks_t = TBL_PAIR_CAP // 128
    in_maps = []
    for k in range(CORES):
        c = cores[k]
        e_grp = grp_of_win[c["e_slot"] // WIN]
        rid = np.zeros(len(c["e_col"]), dtype=np.int64)
        tbls = []
        for g in range(NGRP):
            m = e_grp == g
            uniq, inv = np.unique(c["e_col"][m], return_inverse=True)
            n_ids = len(uniq)
            assert n_ids <= 2 * TBL_PAIR_CAP, f"table overflow: {n_ids}"
            rid[m] = inv
            tbl = np.zeros((128, ranks_t, 2, 64), dtype=ml_dtypes.bfloat16)
            ids = np.arange(n_ids)
            pair = ids >> 1
            tbl[pair & 127, pair >> 7, ids & 1, :] = h_bf[uniq]
            tbls.append(tbl.reshape(128, ranks_t * 128))

        gidx_flat = np.zeros(T * 128, dtype=np.int64)
        rv = np.zeros((128, T * 64), dtype=np.float32)
        tiles_by_win = {}
        for t in range(T):
            tiles_by_win.setdefault(int(tile_win[t]), []).append(t)
        for w in range(nwin):
            elo, ehi = int(c["win_lo"][w]), int(c["win_hi"][w])
            ecount = ehi - elo
            for j, t in enumerate(tiles_by_win.get(w, [])):
                n = max(0, min(128, ecount - j * 128))
                if n <= 0:
                    continue
                sl = slice(elo + j * 128, elo + j * 128 + n)
                ids_e = rid[sl]
                gidx_flat[t * 128: t * 128 + n] = ids_e >> 1
                m_local = (c["e_slot"][sl] % WIN).astype(np.int64)
                parity = (ids_e & 1).astype(np.int64)
                rv[np.arange(n), t * 64 + parity * 32 + m_local] = c["e_val"][sl]

        gidx = np.zeros((128, T * 8), dtype=np.int16)
        off_t = 0
        off_c = 0
        for (g, ntl) in batches:
            n_idx = ntl * 128
            blk = gidx_flat[off_t * 128: off_t * 128 + n_idx]
            wrapped = blk.reshape(n_idx // 16, 16).T.astype(np.int16)
            for r in range(8):
                gidx[16 * r:16 * (r + 1), off_c: off_c + n_idx // 16] = wrapped
            off_t += ntl
            off_c += n_idx // 16

        gl = c["perm"] + k * npc
        valid = c["perm"] < npc
        h0p = np.zeros((128, nblocks * 64), dtype=np.float32)
        slot_idx = np.arange(npc_pad)
        vs = slot_idx[valid]
        h0p[(vs % 128)[:, None],
            ((vs // 128) * 64)[:, None] + np.arange(64)[None, :]] = h0[gl[valid]]

        im = {f"tbl{g}": tbls[g] for g in range(NGRP)}
        im["gidx"] = gidx
        im["rv"] = rv.astype(ml_dtypes.bfloat16)
        im["h0p"] = h0p
        im["ident"] = np.eye(128, dtype=ml_dtypes.bfloat16)
        in_maps.append(im)

    meta = dict(T=T, batches=batches, tile_win=tile_win, nblocks=nblocks,
                npc=npc, npc_pad=npc_pad, ranks_t=ranks_t,
                perms=[c["perm"] for c in cores])
    return in_maps, meta


# ------------------------------------------------------------- graph builder
def _build(meta):
    T = meta["T"]
    batches = meta["batches"]
    tile_win = meta["tile_win"]
    nblocks = meta["nblocks"]
    ranks_t = meta["ranks_t"]
    bf16 = mybir.dt.bfloat16
    f32 = mybir.dt.float32

    nc = bacc.Bacc("TRN2")
    tbl_hbm = [nc.declare_dram_parameter(f"tbl{g}", [128, ranks_t * 128], bf16,
                                         isOutput=False) for g in range(NGRP)]
    gidx_hbm = nc.declare_dram_parameter("gidx", [128, T * 8], mybir.dt.int16,
                                         isOutput=False)
    rv_hbm = nc.declare_dram_parameter("rv", [128, T * 64], bf16, isOutput=False)
    h0p_hbm = nc.declare_dram_parameter("h0p", [128, nblocks * 64], f32,
                                        isOutput=False)
    ident_hbm = nc.declare_dram_parameter("ident", [128, 128], bf16,
                                          isOutput=False)
    out_hbm = nc.declare_dram_parameter("out", [128, nblocks * 64], f32,
                                        isOutput=True)

    from contextlib import ExitStack
    with ExitStack() as ctx:
        block = ctx.enter_context(nc.Block())
        tblb0 = ctx.enter_context(nc.sbuf_tensor("tblb0", [128, ranks_t * 128], bf16))
        tblb1 = ctx.enter_context(nc.sbuf_tensor("tblb1", [128, ranks_t * 128], bf16))
        gbuf0 = ctx.enter_context(nc.sbuf_tensor("gbuf0", [128, 1, BATCH_TILES * 128], bf16))
        gbuf1 = ctx.enter_context(nc.sbuf_tensor("gbuf1", [128, 1, BATCH_TILES * 128], bf16))
        arena0 = ctx.enter_context(nc.sbuf_tensor("arena0", [128, BATCH_TILES, 128], bf16))
        arena1 = ctx.enter_context(nc.sbuf_tensor("arena1", [128, BATCH_TILES, 128], bf16))
        rvb0 = ctx.enter_context(nc.sbuf_tensor("rvb0", [128, BATCH_TILES * 64], bf16))
        rvb1 = ctx.enter_context(nc.sbuf_tensor("rvb1", [128, BATCH_TILES * 64], bf16))
        gidxb = ctx.enter_context(nc.sbuf_tensor("gidxb", [128, T * 8], mybir.dt.int16))
        h0s = ctx.enter_context(nc.sbuf_tensor("h0s", [128, nblocks * 64], f32))
        stage = ctx.enter_context(nc.sbuf_tensor("stage", [128, nblocks * 64], f32))
        identb = ctx.enter_context(nc.sbuf_tensor("identb", [128, 128], bf16))
        pst0 = ctx.enter_context(nc.psum_tensor("pst0", [128, 128], bf16))
        pst1 = ctx.enter_context(nc.psum_tensor("pst1", [128, 128], bf16))
        pso0 = ctx.enter_context(nc.psum_tensor("pso0", [128, 512], f32))
        pso1 = ctx.enter_context(nc.psum_tensor("pso1", [128, 512], f32))
        pso2 = ctx.enter_context(nc.psum_tensor("pso2", [128, 512], f32))
        pso3 = ctx.enter_context(nc.psum_tensor("pso3", [128, 512], f32))
        s_in = ctx.enter_context(nc.semaphore("s_in"))
        s_tbl = [ctx.enter_context(nc.semaphore("s_tbl0")),
                 ctx.enter_context(nc.semaphore("s_tbl1"))]
        s_rv = [ctx.enter_context(nc.semaphore("s_rv0")),
                ctx.enter_context(nc.semaphore("s_rv1"))]
        s_ga = [ctx.enter_context(nc.semaphore("s_ga0")),
                ctx.enter_context(nc.semaphore("s_ga1"))]
        s_tp = ctx.enter_context(nc.semaphore("s_tp"))
        s_ev = ctx.enter_context(nc.semaphore("s_ev"))
        s_rd = ctx.enter_context(nc.semaphore("s_rd"))
        s_ae = ctx.enter_context(nc.semaphore("s_ae"))
        s_h0 = ctx.enter_context(nc.semaphore("s_h0"))
        tblb = [tblb0, tblb1]
        gbuf = [gbuf0, gbuf1]
        arena = [arena0, arena1]
        rvb = [rvb0, rvb1]
        pso = [pso0, pso1, pso2, pso3]
        NSLOT = 32

        bt_tiles = []
        t0 = 0
        for (g, ntl) in batches:
            bt_tiles.append((g, list(range(t0, t0 + ntl))))
            t0 += ntl
        cum_mm = []
        acc = 0
        for (g, ntl) in batches:
            acc += 2 * ntl
            cum_mm.append(acc)

        win_mm_count = {}
        for t in range(T):
            w = int(tile_win[t])
            win_mm_count[w] = win_mm_count.get(w, 0) + 2
        win_mm_seen = {w: 0 for w in win_mm_count}

        blk_last_mm = {}
        mm_i = 0
        for t in range(T):
            mm_i += 2
            blk_last_mm[int(tile_win[t]) // 4] = mm_i

        # first batch index of each group (for table prefetch scheduling)
        first_batch_of_grp = {}
        for i, (g, _) in enumerate(batches):
            first_batch_of_grp.setdefault(g, i)
        batches_until_grp = {g: sum(1 for (gg, _) in batches if gg <= g - 2)
                             for g in range(NGRP)}

        # ---- sync: streams in dependency order, then final store
        @block.sync
        def _(s):
            s.dma_start(gidxb[:], gidx_hbm[:]).then_inc(s_in, 16)
            s.dma_start(h0s[:], h0p_hbm[:]).then_inc(s_in, 16)
            s.dma_start(identb[:], ident_hbm[:]).then_inc(s_in, 16)
            s.dma_start(tblb[0][:], tbl_hbm[0][:]).then_inc(s_tbl[0], 16)
            s.dma_start(tblb[1][:], tbl_hbm[1][:]).then_inc(s_tbl[1], 16)
            loaded_grp = 2
            off = 0
            for i, (g, tl) in enumerate(bt_tiles):
                ntl = len(tl)
                if loaded_grp < NGRP and i == first_batch_of_grp.get(loaded_grp - 2 + 2, -1):
                    pass
                # prefetch table for group loaded_grp as soon as gathers of
                # group loaded_grp-2 are done
                if loaded_grp < NGRP and g == loaded_grp - 1:
                    nb = batches_until_grp[loaded_grp]
                    for ib in (nb - 1, nb - 2):
                        if ib >= 0:
                            s.wait_ge(s_ga[ib % 2], 16 * (ib // 2 + 1))
                    s.dma_start(tblb[loaded_grp % 2][:],
                                tbl_hbm[loaded_grp][:]).then_inc(
                                    s_tbl[loaded_grp % 2], 16)
                    loaded_grp += 1
                if i >= 2:
                    s.wait_ge(s_rd, cum_mm[i - 2])
                s.dma_start(rvb[i % 2][:, 0:ntl * 64],
                            rv_hbm[:, off * 64:(off + ntl) * 64]).then_inc(
                                s_rv[i % 2], 16)
                off += ntl
            while loaded_grp < NGRP:
                nb = batches_until_grp[loaded_grp]
                for ib in (nb - 1, nb - 2):
                    if ib >= 0:
                        s.wait_ge(s_ga[ib % 2], 16 * (ib // 2 + 1))
                s.dma_start(tblb[loaded_grp % 2][:],
                            tbl_hbm[loaded_grp][:]).then_inc(
                                s_tbl[loaded_grp % 2], 16)
                loaded_grp += 1
            s.wait_ge(s_h0, 1 + nblocks)
            s.dma_start(out_hbm[:], stage[:]).then_inc(s_in, 16)
            s.wait_ge(s_in, 64)

        # ---- gpsimd: gathers
        @block.gpsimd
        def _(ge: bass.BassGpSimd):
            ge.load_library(mlp)
            ge.wait_ge(s_in, 48)
            col = 0
            for i, (g, tl) in enumerate(bt_tiles):
                n_idx = len(tl) * 128
                ge.wait_ge(s_tbl[g % 2], 16 * (g // 2 + 1))
                if i >= 2:
                    ge.wait_ge(s_rd, cum_mm[i - 2])
                ge.dma_gather(
                    gbuf[i % 2][:, :, 0:n_idx],
                    tblb[g % 2][:],
                    gidxb[:, col:col + n_idx // 16],
                    n_idx, n_idx, 128,
                    transpose=True, single_packet=False,
                    sbuf_tokens_per_rank=TPR,
                    sbuf_free_dim_per_rank=256,
                    sbuf_free_dim_pad_per_rank=0,
                    sbuf_byte_offset=0,
                ).then_inc(s_ga[i % 2], 16)
                col += n_idx // 16

        # ---- tensor: transposes + reductions
        @block.tensor
        def _(te):
            te.wait_ge(s_in, 48)
            tp = 0
            for i, (g, tl) in enumerate(bt_tiles):
                te.wait_ge(s_ga[i % 2], 16 * (i // 2 + 1))
                te.wait_ge(s_rv[i % 2], 16 * (i // 2 + 1))
                for j, t in enumerate(tl):
                    if tp >= 2:
                        te.wait_ge(s_ev, tp - 1)
                    te.transpose(
                        out=(pst0 if tp % 2 == 0 else pst1)[:],
                        in_=gbuf[i % 2][:, 0, j * 128:(j + 1) * 128],
                        identity=identb[:],
                    ).then_inc(s_tp, 1)
                    tp += 1
                    w = int(tile_win[t])
                    blk, q = w // 4, w % 4
                    slot = blk % NSLOT
                    bank = pso[slot // 8]
                    c0 = (slot % 8) * 64
                    te.wait_ge(s_ev, tp)
                    if blk >= NSLOT:
                        te.wait_ge(s_ae, blk - NSLOT + 1)
                    first = win_mm_seen[w] == 0
                    win_mm_seen[w] += 2
                    last = win_mm_seen[w] == win_mm_count[w]
                    te.matmul(
                        out=bank[32 * q:32 * q + 32, c0:c0 + 64],
                        lhsT=rvb[i % 2][:, j * 64:j * 64 + 32],
                        rhs=arena[i % 2][:, j, 0:64],
                        start=first, stop=False,
                        tile_position=(0, 32 * q), skip_group_check=True,
                    ).then_inc(s_rd, 1)
                    mm2 = te.matmul(
                        out=bank[32 * q:32 * q + 32, c0:c0 + 64],
                        lhsT=rvb[i % 2][:, j * 64 + 32:j * 64 + 64],
                        rhs=arena[i % 2][:, j, 64:128],
                        start=False, stop=last,
                        tile_position=(0, 32 * q), skip_group_check=True,
                    ).then_inc(s_rd, 1)


        # ---- vector: h0 prescale, arena evictions, block h0-adds
        @block.vector
        def _(v):
            v.wait_ge(s_in, 48)
            v.tensor_scalar(out=h0s[:], in0=h0s[:], scalar1=ALPHA, scalar2=None,
                            op0=mybir.AluOpType.mult).then_inc(s_h0, 1)
            tp = 0
            for i, (g, tl) in enumerate(bt_tiles):
                for j, t in enumerate(tl):
                    v.wait_ge(s_tp, tp + 1)
                    v.tensor_copy(
                        out=arena[i % 2][:, j, :],
                        in_=(pst0 if tp % 2 == 0 else pst1)[:],
                    ).then_inc(s_ev, 1)
                    tp += 1
            for b in range(nblocks):
                v.wait_ge(s_ae, b + 1)
                v.tensor_tensor(
                    out=stage[:, b * 64:(b + 1) * 64],
                    in0=stage[:, b * 64:(b + 1) * 64],
                    in1=h0s[:, b * 64:(b + 1) * 64],
                    op=mybir.AluOpType.add,
                ).then_inc(s_h0, 1)

        # ---- scalar: block evictions (0.8 * psum)
        @block.scalar
        def _(sc):
            for b in range(nblocks):
                sc.wait_ge(s_rd, blk_last_mm.get(b, 0))
                slot = b % NSLOT
                bank = pso[slot // 8]
                c0 = (slot % 8) * 64
                sc.activation(
                    out=stage[:, b * 64:(b + 1) * 64],
                    in_=bank[:, c0:c0 + 64],
                    func=mybir.ActivationFunctionType.Copy,
                    scale=1.0 - ALPHA,
                ).then_inc(s_ae, 1)

    nc.compile()
    return nc


_CACHE = {}


def kernel(edge_row, edge_col, edge_vals, h, h0):
    in_maps, meta = _preprocess(edge_row, edge_col, edge_vals, h, h0)
    key = (meta["T"], tuple(meta["batches"]))
    if key not in _CACHE:
        _CACHE[key] = _build(meta)
    nc = _CACHE[key]
    res = bass_utils.run_bass_kernel_spmd(nc, in_maps, core_ids=list(range(CORES)))
    npc = meta["npc"]
    out = np.zeros((N_NODES, D), dtype=np.float32)
    for k in range(CORES):
        o = np.asarray(res.results[k]["out"], dtype=np.float32)
        o = o.reshape(128, meta["nblocks"], D).transpose(1, 0, 2).reshape(-1, D)
        perm = meta["perms"][k]
        valid = perm < npc
        out[perm[valid] + k * npc] = o[valid]
    return out
